# revision 20
# baseline (speedup 1.0000x reference)
"""Trainium2 Bass kernel for nn_BiEncoder_63024350101542 (segment_reduce).

Reference, per batch row b of vector_all [B=64, L=512, D=1024]:
    mask[b,j] = (j > first_idx(ids[b]==1)) & (j < first_idx(ids[b]==2))
    span_max  = max over masked rows (fallback: CLS row 0 when mask empty)
    out[b]    = cls + mu * span_max

Only rows inside the mention span (plus CLS for empty spans) can affect
the output, so the host packs exactly those rows: batches are
balance-assigned across the 8 NeuronCores (8 per core), each batch's
rows are padded to a multiple of 32 with -1e30 filler and concatenated
into T tiles of 128 rows. The 32-row alignment means the DVE
transpose-fused reduce's natural 32-partition groups never straddle
batches, so ONE reduce per tile computes all group maxima; a small
uploaded group-ownership bias matrix (0 / -1e30) then routes groups to
output slots. T adapts to the actual inputs each call, so the kernel
stays fully general (worst case ~ full streaming).

Device pipeline per tile t (incremental, overlapped with the DMA):
  S_t          = ttr-max(x[t])                      # [128,32] group maxima
  V[:, i, :]   = max(V[:, i, :], S_t + b2[:, t, i]) # slot-select accumulate
                  (2 slots fused on DVE, 6 via ACT bias-add + Pool max)
Tail: PE transpose + max over the 4 partition groups, cls + mu * vec,
one output DMA. Tiles stream on two hardware DGE queues (sync+scalar);
constants ride the gpsimd queue.
"""

import os
import sys

import numpy as np

for _p in ("/root/.axon_site/_ro/trn_rl_repo", "/opt/trn_rl_repo"):
    if _p not in sys.path and os.path.isdir(_p):
        sys.path.append(_p)

import concourse.bacc as bacc
import concourse.bass as bass
import concourse.mybir as mybir
import concourse.tile as tile
from concourse.bass_utils import run_bass_kernel_spmd

F32 = mybir.dt.float32
X = mybir.AxisListType.X
Alu = mybir.AluOpType
Act = mybir.ActivationFunctionType

B, L, D = 64, 512, 1024
NCORES = 8
SLOTS = B // NCORES        # batches (output slots) per core
BIG = 1.0e30
SEL_DVE = 2                # slots whose select runs fused on DVE


def build_bass(T: int, H: int = 128):
    nc = bacc.Bacc("TRN2", target_bir_lowering=False, debug=False)

    CW = T * SLOTS + 128 + 1 + 64   # b2 | identity | mu | cls2
    xd = nc.dram_tensor("xpack", [T, 128, D], F32, kind="ExternalInput").ap()
    consts = nc.dram_tensor("consts", [128, CW], F32, kind="ExternalInput").ap()
    out = nc.dram_tensor("out", [SLOTS, D], F32, kind="ExternalOutput").ap()

    with tile.TileContext(nc) as tc:
        with (
            tc.tile_pool(name="persist", bufs=1) as pp,
            tc.tile_pool(name="acc", bufs=2) as vpool,
            tc.tile_pool(name="stage", bufs=4) as spool,
            tc.tile_pool(name="tr", bufs=2, space="PSUM") as ppool,
        ):
            # tile 0 solo on sync for the fastest ramp; scalar does the
            # one-shot constants blob, then odd tiles
            x_sb = pp.tile([128, T, D], F32)
            nc.sync.dma_start(out=x_sb[:, 0, :], in_=xd[0])

            c_sb = pp.tile([128, CW], F32)
            nc.scalar.dma_start(out=c_sb[:], in_=consts)
            b2_sb = c_sb[:, 0 : T * SLOTS].rearrange(
                "p (t s) -> p t s", s=SLOTS
            )
            ident_sb = c_sb[:, T * SLOTS : T * SLOTS + 128]
            mu_col = c_sb[:, T * SLOTS + 128 : T * SLOTS + 129]
            cls_sb = c_sb[:, T * SLOTS + 129 : CW].rearrange(
                "p (blk ii) -> p blk ii", blk=2
            )

            for t in range(1, T):
                eng = nc.sync if t % 2 == 0 else nc.scalar
                ht = H if t == T - 1 else 128
                eng.dma_start(out=x_sb[0:ht, t, :], in_=xd[t][0:ht, :])

            # S_t[32a+ii, m] = max_{p in [32a,32a+32)} x[p, t, 32m+ii]
            S = pp.tile([128, T, 32], F32)
            V = vpool.tile([128, SLOTS, 32], F32, tag="V")
            terms = [None] * T
            for t in range(T - 1):
                nc.vector.tensor_reduce(
                    S[:, t, :],
                    x_sb[:, t, :].rearrange("p (m c) -> p m c", c=32),
                    axis=X, op=Alu.max, apply_transpose=True,
                )
                # slot-select accumulate: V[:,i,:] = max(V[:,i,:], S_t + b2[:,t,i])
                Vn = V if t == 0 else vpool.tile([128, SLOTS, 32], F32, tag="V")
                for i in range(SEL_DVE):
                    if t == 0:
                        nc.vector.tensor_scalar(
                            out=Vn[:, i, :], in0=S[:, 0, :],
                            scalar1=b2_sb[:, 0, i : i + 1], scalar2=None,
                            op0=Alu.add,
                        )
                    else:
                        nc.vector.scalar_tensor_tensor(
                            out=Vn[:, i, :], in0=S[:, t, :],
                            scalar=b2_sb[:, t, i : i + 1],
                            in1=V[:, i, :], op0=Alu.add, op1=Alu.max,
                        )
                nact = SLOTS - SEL_DVE
                tgt = Vn[:, SEL_DVE:, :] if t == 0 else None
                if t > 0:
                    trm = spool.tile([128, nact, 32], F32, tag="terms")
                    terms[t] = trm
                for k, i in enumerate(range(SEL_DVE, SLOTS)):
                    dst = tgt[:, k, :] if t == 0 else terms[t][:, k, :]
                    nc.scalar.activation(
                        dst, S[:, t, :], Act.Identity,
                        bias=b2_sb[:, t, i : i + 1], scale=1.0,
                    )
                if t > 0:
                    nc.vector.tensor_tensor(
                        out=Vn[:, SEL_DVE:, :], in0=V[:, SEL_DVE:, :],
                        in1=terms[t][:], op=Alu.max,
                    )
                V = Vn

            # last tile: partial height; all-DVE selects with the PE
            # block transposes interleaved to shorten the tail
            t = T - 1
            if H < 128:
                nc.vector.memset(S[H:, t, :], -BIG)
            nc.vector.tensor_reduce(
                S[0:H, t, :],
                x_sb[0:H, t, :].rearrange("p (m c) -> p m c", c=32),
                axis=X, op=Alu.max, apply_transpose=True,
            )
            Vn = V if T == 1 else vpool.tile([128, SLOTS, 32], F32, tag="V")
            VT = ppool.tile([128, 2, 128], F32, tag="VT")
            for blk in range(2):
                for i in range(4 * blk, 4 * blk + 4):
                    if T == 1:
                        nc.vector.tensor_scalar(
                            out=Vn[:, i, :], in0=S[:, t, :],
                            scalar1=b2_sb[:, t, i : i + 1], scalar2=None,
                            op0=Alu.add,
                        )
                    else:
                        nc.vector.scalar_tensor_tensor(
                            out=Vn[:, i, :], in0=S[:, t, :],
                            scalar=b2_sb[:, t, i : i + 1],
                            in1=V[:, i, :], op0=Alu.add, op1=Alu.max,
                        )
                nc.tensor.transpose(
                    VT[:, blk, :],
                    Vn[:].rearrange("p (blk i4) m -> p blk (i4 m)", blk=2)[:, blk, :],
                    ident_sb[:],
                )
            fin = spool.tile([128, 2, 32], F32, tag="fin")
            nc.vector.tensor_reduce(
                fin[:], VT[:].rearrange("p blk (a ii) -> p blk ii a", a=4),
                axis=X, op=Alu.max,
            )

            # out = cls + mu * vec   (partition c' = 32*i4+m; col = blk, ii)
            oT = spool.tile([128, 2, 32], F32, tag="oT")
            nc.vector.scalar_tensor_tensor(
                out=oT[:], in0=fin[:], scalar=mu_col[:, 0:1],
                in1=cls_sb[:], op0=Alu.mult, op1=Alu.add,
            )
            nc.gpsimd.dma_start(
                out=out.rearrange("(blk i4) (m ii) -> (i4 m) blk ii", blk=2, m=32),
                in_=oT[:],
            )

    nc.compile()
    return nc


def plan_packing(ids: np.ndarray):
    """Host-side span + packing plan (pure index math on ids).

    Returns (assign, row_lists, T):
      assign[c][i] = global batch index of core c, slot i
      row_lists[b] = contributing row indices of batch b
                     (span rows, or [0] when the span is empty)
    """
    Bc, Lc = ids.shape
    is1 = ids == 1
    is2 = ids == 2
    first1 = np.where(is1.any(1), is1.argmax(1), Lc)
    first2 = np.where(is2.any(1), is2.argmax(1), Lc)
    row_lists = []
    for b in range(Bc):
        lo, hi = int(first1[b]) + 1, min(int(first2[b]), Lc)
        rows = list(range(lo, hi)) or [0]
        row_lists.append(rows)

    aligned = [((len(r) + 31) // 32) * 32 for r in row_lists]
    order = sorted(range(Bc), key=lambda b: -aligned[b])
    loads = [0] * NCORES
    assign = [[] for _ in range(NCORES)]
    for b in order:
        c = min(
            (c for c in range(NCORES) if len(assign[c]) < SLOTS),
            key=lambda c: loads[c],
        )
        assign[c].append(b)
        loads[c] += aligned[b]

    # pairwise-swap refinement to shave the max load
    for _ in range(200):
        hi = max(range(NCORES), key=lambda c: loads[c])
        best = None
        for lo in range(NCORES):
            if lo == hi:
                continue
            for bi, bh in enumerate(assign[hi]):
                for bj, bl in enumerate(assign[lo]):
                    d = aligned[bh] - aligned[bl]
                    if d <= 0:
                        continue
                    new_hi = loads[hi] - d
                    new_lo = loads[lo] + d
                    if max(new_hi, new_lo) < loads[hi] and (
                        best is None or max(new_hi, new_lo) < best[0]
                    ):
                        best = (max(new_hi, new_lo), lo, bi, bj)
        if best is None:
            break
        _, lo, bi, bj = best
        bh, bl = assign[hi][bi], assign[lo][bj]
        assign[hi][bi], assign[lo][bj] = bl, bh
        loads[hi] += aligned[bl] - aligned[bh]
        loads[lo] += aligned[bh] - aligned[bl]

    max_load = max(max(loads), 32)
    T = (max_load + 127) // 128
    H = max_load - 128 * (T - 1)       # partial height of the last tile
    return assign, row_lists, T, H


def make_in_maps(vector_all, ids, mu):
    va = np.ascontiguousarray(np.asarray(vector_all, dtype=np.float32))
    ids = np.ascontiguousarray(np.asarray(ids, dtype=np.int32))
    assign, row_lists, T, H = plan_packing(ids)

    mu_col = np.full(
        (128, 1), np.asarray(mu, dtype=np.float32).reshape(-1)[0],
        dtype=np.float32,
    )
    ident = np.eye(128, dtype=np.float32)
    CW = T * SLOTS + 128 + 1 + 64

    in_maps = []
    for c in range(NCORES):
        xpack = np.full((T, 128, D), -BIG, dtype=np.float32)
        b2 = np.full((128, T, SLOTS), -BIG, dtype=np.float32)
        cls2 = np.empty((128, 2, 32), dtype=np.float32)
        j = 0
        for i, b in enumerate(assign[c]):
            rows = row_lists[b]
            pos = np.arange(j, j + len(rows))
            xpack[pos // 128, pos % 128, :] = va[b, rows, :]
            # groups this batch owns: [j/32, ceil((j+len)/32))
            g0, g1 = j // 32, (j + len(rows) + 31) // 32
            for g in range(g0, g1):
                t, a = g // 4, g % 4
                b2[32 * a : 32 * a + 32, t, i] = 0.0
            # cls in the output layout: partition 32*i4+m, cols (blk, ii)
            blk, i4 = i // 4, i % 4
            cls2[32 * i4 : 32 * i4 + 32, blk, :] = va[b, 0, :].reshape(32, 32)
            j += ((len(rows) + 31) // 32) * 32
        consts = np.concatenate(
            [
                b2.reshape(128, T * SLOTS),
                ident,
                mu_col,
                cls2.reshape(128, 64),
            ],
            axis=1,
        ).astype(np.float32)
        assert consts.shape == (128, CW)
        in_maps.append({"xpack": xpack, "consts": consts})
    return in_maps, assign, T, H


def run(vector_all, ids, mu, trace=False):
    """Returns (out [B, D] f32, BassKernelResults)."""
    in_maps, assign, T, H = make_in_maps(vector_all, ids, mu)
    nc = build_bass(T, H)
    res = run_bass_kernel_spmd(nc, in_maps, list(range(NCORES)), trace=trace)
    out = np.empty((B, D), dtype=np.float32)
    for c in range(NCORES):
        out[assign[c]] = res.results[c]["out"]
    return out, res


def kernel(**inputs) -> np.ndarray:
    out, _ = run(inputs["vector_all"], inputs["ids"], inputs["mu"])
    return out
